# revision 25
# baseline (speedup 1.0000x reference)
"""GraphSAGE layer kernel for Trainium2, SPMD over 8 NeuronCores.

Math (per reference):
    x3   = inputs.reshape(B, N, D)                      # B=128, N=4096, D=32
    out  = relu(x3 @ W_self + (A^T @ (x3 @ W_neigh)))   # per batch
    out  = out.reshape(B, N*D)

Strategy (v4: fp8 DoubleRow aggregation + self-part on the PE):
  - Pure data-parallel over batch: 16 batches per core. Each core streams
    the full adjacency A as matmul stationary operands.
  - XT layout [128 partitions = (b%4)*32 + p, (iblk, b//4, i%128)] fp16
    stays RESIDENT in SBUF (8 chunk tiles) and is read twice by the PE:
    once by the transform (T = X@Wn via a [128,128] block-diagonal Wn as
    moving operand) and once per output j-block by the self-part matmuls.
  - T is evacuated PSUM->SBUF as fp8e4 (it only feeds the small
    neighbor-aggregation term: ~3.6% quantization of a ~1.8%-of-output
    component), one [128,16,32] copy per i-block, alternating DVE/ACT.
  - Aggregation per j-block: psum[j,(b,q)] = SC*self + SC*neigh:
      * 4 fp16 matmuls: xt[jb,b4] (stationary) @ block-diag(Ws*SC) accumulate
        the self part (full fp16/fp32 precision, scaled by SC=2^12).
      * 16 fp8 DoubleRow matmuls: stationary [128,2,128] = A pair-blocks
        (host-pretransposed [i%128,(jb,ib,j%128)], scaled by SC so A's
        ~2.4e-4 entries sit in e4m3's normal range), moving [128,2,512] =
        T pair-blocks — 0.5 PE cycles/row, 4x the fp16 rate.
  - Evacuation: one relu(psum * 1/SC) per j-block (relu(x)/SC ==
    relu(x/SC)), alternating ACT activation / DVE tensor_scalar, written
    fp16 and DMAed to y[j, b_local, q]; host untransposes + casts fp32.
  - Queue discipline: Pool/SWDGE issues the big loads strictly ordered
    (9 XT chunks — the first two half-size so the PE starts earlier —
    then 32 A panels) so XT is never stuck behind A on the serialized
    DMA engines; sync/SP issues weight + Y DMAs. The A ring stays at 7
    tiles: more in-flight Pool DMAs would overflow the 1024-descriptor
    SWDGE carveout and head-block the Pool queue. The first 4 j-blocks'
    self parts are pre-accumulated into held psum banks during the
    transform, filling PE stalls while late xt chunks stream in.
"""

import numpy as np

B, N, D = 128, 4096, 32
NCORES = 8
BSH = B // NCORES          # 16 batches per core
NIB = N // 128             # 32 node blocks
NB4 = BSH // 4             # 4 groups of 4 batches
BQ = BSH * D               # 512 = moving free width of big matmul
SC = 4096.0                # fp8 scale for A and the self part

_CACHE = {}


def _build_program():
    import concourse.bacc as bacc
    import concourse.mybir as mybir
    import concourse.tile as tile
    from contextlib import ExitStack

    f32 = mybir.dt.float32
    fp16 = mybir.dt.float16
    fp8 = mybir.dt.float8e4
    DR = mybir.MatmulPerfMode.DoubleRow
    Relu = mybir.ActivationFunctionType.Relu
    Alu = mybir.AluOpType

    nc = bacc.Bacc(
        trn_type="TRN2", target_bir_lowering=False, debug=False, num_devices=NCORES
    )
    xt = nc.dram_tensor("xt", [128, NB4 * N], fp16, kind="ExternalInput").ap()
    bdn = nc.dram_tensor("bdn", [128, 128], fp16, kind="ExternalInput").ap()
    bds = nc.dram_tensor("bds", [128, 128], fp16, kind="ExternalInput").ap()
    a = nc.dram_tensor("a", [128, NIB * NIB * 128], fp8, kind="ExternalInput").ap()
    y = nc.dram_tensor("y", [N, BQ], fp16, kind="ExternalOutput").ap()

    with tile.TileContext(nc) as tc, ExitStack() as ctx:
        const_pool = ctx.enter_context(tc.tile_pool(name="const", bufs=1))
        xt_pool = ctx.enter_context(tc.tile_pool(name="xtp", bufs=8))
        t_pool = ctx.enter_context(tc.tile_pool(name="tp", bufs=1))
        a_pool = ctx.enter_context(tc.tile_pool(name="ap", bufs=7))
        out_pool = ctx.enter_context(tc.tile_pool(name="op", bufs=4))
        pt_pool = ctx.enter_context(tc.tile_pool(name="ptp", bufs=4, space="PSUM"))
        po_pool = ctx.enter_context(tc.tile_pool(name="pop", bufs=4, space="PSUM"))

        bdn_sb = const_pool.tile([128, 128], fp16)
        bds_sb = const_pool.tile([128, 128], fp16)
        nc.sync.dma_start(bdn_sb[:], bdn[:])

        # T as 16 pair-tensors [i%128, (2, b, qn)] fp8 so each DoubleRow
        # pair matmul depends only on its own two evacuations (tile-level
        # dependency tracking would otherwise serialize the weave on all 32)
        t_tiles = [
            t_pool.tile([128, 2, BSH, D], fp8, name=f"t{k}") for k in range(NIB // 2)
        ]

        # a is host-pretransposed: a[ip, (jb, ib, jj)], scaled by SC
        a_r = a.rearrange("p (jb ib jj) -> p jb ib jj", jb=NIB, ib=NIB)
        # xt is host-laid-out ib-major: xt[(bh,p), (ib, b4, il)]
        xt_r = xt.rearrange("p (ib b4 il) -> p ib b4 il", ib=NIB, b4=NB4)

        # ---- transform: T via block-diag Wn; xt chunks stay resident ----
        # first chunk is half-size so the PE starts ~0.7us earlier
        chunk_sizes = [2, 2] + [4] * 7
        xt_by_ib = {}   # ib -> (xt_tile, local index)
        po_tiles = {}   # jb -> psum tile with the self part pre-accumulated

        def emit_self(jb, po):
            xt_t, ibl = xt_by_ib[jb]
            for b4 in range(NB4):
                nc.tensor.matmul(
                    po[:, b4 * 4 : (b4 + 1) * 4, :],
                    xt_t[:, ibl, b4, :], bds_sb[:],
                    start=(b4 == 0), stop=False,
                )

        NEARLY = 4          # j-blocks whose aggregation weaves into transform
        a_tiles = {}
        pairs_done = {k: 0 for k in range(NEARLY)}

        def emit_dr(jb, po, a_t, p):
            nc.tensor.matmul(
                po[:],
                a_t[:, p : p + 2, :],
                t_tiles[p // 2][:],
                start=False,
                stop=(p == NIB - 2),
                perf_mode=DR,
            )

        ib = 0
        for c, sz in enumerate(chunk_sizes):
            xt_t = xt_pool.tile([128, sz, NB4, 128], fp16, tag="xt", name=f"xt{c}")
            nc.gpsimd.dma_start(xt_t[:], xt_r[:, ib : ib + sz, :, :])
            if c == 0:
                nc.sync.dma_start(bds_sb[:], bds[:])
            for ibl in range(sz):
                xt_by_ib[ib] = (xt_t, ibl)
                pt = pt_pool.tile([128, NB4, 128], f32, tag="pt", name=f"pt{ib}")
                for b4 in range(NB4):
                    nc.tensor.matmul(
                        pt[:, b4, :], xt_t[:, ibl, b4, :], bdn_sb[:],
                        start=True, stop=True,
                    )
                # pt[p, b4, (bh, qn)] -> t[p, ib, (b4 bh), qn]
                ptv = pt.rearrange("p b4 (bh q) -> p (b4 bh) q", bh=4)
                tdst = t_tiles[ib // 2][:, ib % 2, :, :]
                if ib % 2 == 0:
                    nc.vector.tensor_copy(tdst, ptv[:])
                else:
                    nc.scalar.copy(tdst, ptv[:])
                ib += 1
            if c == 1:
                # xt for j-blocks 0..3 is now resident: pre-accumulate their
                # self parts into held psum banks
                for jb in range(NEARLY):
                    po = po_pool.tile([128, BSH, D], f32, tag="po", name=f"po{jb}")
                    po_tiles[jb] = po
                    emit_self(jb, po)
        # ---- aggregation: psum[j,(b,q)] = SC*self + SC*neigh ----
        for jb in range(NIB):
            a_t = a_pool.tile([128, NIB, 128], fp8, tag="a", name=f"a{jb}")
            nc.gpsimd.dma_start(a_t[:], a_r[:, jb, :, :])
            po = po_tiles.pop(jb, None)
            if po is None:
                po = po_pool.tile([128, BSH, D], f32, tag="po", name=f"po{jb}")
                emit_self(jb, po)
            for ib2 in range(0, NIB, 2):
                emit_dr(jb, po, a_t, ib2)
            yb = out_pool.tile([128, BQ], fp16, tag="yb", name=f"yb{jb}")
            pof = po.rearrange("p b q -> p (b q)")
            if jb == NIB - 1:
                # split the last evacuation across both engines and two DMAs
                # to shorten the end-of-kernel latency chain
                nc.scalar.activation(yb[:, 0:256], pof[:, 0:256], Relu,
                                     scale=1.0 / SC)
                nc.vector.tensor_scalar(
                    yb[:, 256:512], pof[:, 256:512], 0.0, 1.0 / SC,
                    op0=Alu.max, op1=Alu.mult,
                )
                nc.sync.dma_start(y[jb * 128 : (jb + 1) * 128, 0:256],
                                  yb[:, 0:256])
                nc.sync.dma_start(y[jb * 128 : (jb + 1) * 128, 256:512],
                                  yb[:, 256:512])
                continue
            if jb % 2 == 0:
                nc.scalar.activation(yb[:], pof, Relu, scale=1.0 / SC)
            else:
                nc.vector.tensor_scalar(
                    yb[:], pof, 0.0, 1.0 / SC, op0=Alu.max, op1=Alu.mult
                )
            nc.sync.dma_start(y[jb * 128 : (jb + 1) * 128, :], yb[:])

    nc.compile()
    return nc


def _get_program():
    if "nc" not in _CACHE:
        _CACHE["nc"] = _build_program()
    return _CACHE["nc"]


def make_in_maps(x3, adj, W_neigh, W_self):
    import ml_dtypes

    # block-diagonal weights, 4 copies along the partition dim:
    # bdn = diag4(W_neigh); bds = diag4(W_self * SC)
    bdn = np.zeros((128, 128), dtype=np.float32)
    bds = np.zeros((128, 128), dtype=np.float32)
    for bh in range(4):
        bdn[bh * 32 : (bh + 1) * 32, bh * 32 : (bh + 1) * 32] = W_neigh
        bds[bh * 32 : (bh + 1) * 32, bh * 32 : (bh + 1) * 32] = W_self * SC
    bdn = bdn.astype(np.float16)
    bds = bds.astype(np.float16)

    # pretranspose A to [ip, (jb, ib, jj)], scale into fp8e4 normal range
    adj_fp8 = (
        np.ascontiguousarray(adj.reshape(NIB, 128, NIB, 128).transpose(1, 2, 0, 3))
        .reshape(128, NIB * NIB * 128) * np.float32(SC)
    ).astype(ml_dtypes.float8_e4m3)

    in_maps = []
    for c in range(NCORES):
        xs = x3[c * BSH : (c + 1) * BSH]          # [16, N, 32]
        # XT[(bh*32+p), (ib, b4, il)] = xs[b4*4 + bh, ib*128 + il, p]
        xtc = np.ascontiguousarray(
            xs.reshape(NB4, 4, NIB, 128, D).transpose(1, 4, 2, 0, 3)
        ).reshape(128, NB4 * N).astype(np.float16)
        in_maps.append({"xt": xtc, "bdn": bdn, "bds": bds, "a": adj_fp8})
    return in_maps


def kernel(inputs, adj, W_neigh, W_self, batch_train=None):
    from concourse.bass_utils import run_bass_kernel_spmd

    inputs = np.asarray(inputs, dtype=np.float32)
    adj = np.ascontiguousarray(np.asarray(adj, dtype=np.float32))
    W_neigh = np.asarray(W_neigh, dtype=np.float32)
    W_self = np.asarray(W_self, dtype=np.float32)

    x3 = inputs.reshape(B, N, D)
    in_maps = make_in_maps(x3, adj, W_neigh, W_self)

    nc = _get_program()
    res = run_bass_kernel_spmd(nc, in_maps, list(range(NCORES)))

    out = np.empty((B, N * D), dtype=np.float32)
    for c in range(NCORES):
        yc = np.asarray(res.results[c]["y"], dtype=np.float32)  # [j, (b_loc, q)]
        out[c * BSH : (c + 1) * BSH] = (
            yc.reshape(N, BSH, D).transpose(1, 0, 2).reshape(BSH, N * D)
        )
    return out
